# revision 1
# baseline (speedup 1.0000x reference)
"""Elman RNN cell (tanh) on 8 Trainium2 NeuronCores.

h_t = tanh(h_{t-1} @ W_h^T + b_h + x_t @ W_x^T + b_x), return h_T.

Strategy (hardcoded for B=64, T=512, I=H=1024, 8 cores):
  - Data parallel over batch: 8 batch elements per core. Weights replicated.
  - Phase 1 (per core): xp[h, t, b] = sum_i W_x[h,i] x[b,t,i] + (b_x+b_h)[h]
    as a tiled matmul with W_x^T stationary (float32r) and x^T moving,
    output written to a DRAM scratch in [p, m, t*8+b] layout (h = m*128+p).
  - Phase 2 (per core): 512 sequential steps. W_h^T stationary in bf16
    (fast weight load), h kept as hT[p, k, b] (h_in = k*128+p) so the
    matmul output [h_out partitions, batch] is directly the next hT.
    psum[p, j, b] += sum_k W_h^T[k-chunk, j-chunk]^T @ hT[:, k, :], then
    hT' = tanh(psum + xp_t) via DVE add + ACT tanh.
"""

import os
import sys

sys.path.insert(0, "/opt/trn_rl_repo")

import numpy as np
import ml_dtypes

import concourse.bass as bass  # noqa: F401
import concourse.tile as tile
from concourse import bacc, mybir
from concourse.bass_utils import run_bass_kernel_spmd
from concourse.tile import TileContext

B, T, I, H = 64, 512, 1024, 1024
N_CORES = 8
BC = B // N_CORES  # batch per core = 8
KI = I // 128      # 8 k-chunks of the input dim
KH = H // 128      # 8 chunks of the hidden dim
F32 = mybir.dt.float32
F32R = mybir.dt.float32r
BF16 = mybir.dt.bfloat16
AF = mybir.ActivationFunctionType

_BUILT = None


def build(t_steps: int = T):
    nc = bacc.Bacc("TRN2", target_bir_lowering=False, debug=False,
                   num_devices=N_CORES)

    # Per-core inputs (host pre-arranged).
    xT = nc.dram_tensor("xT", [I, t_steps * BC], F32R, kind="ExternalInput")
    wxT = nc.dram_tensor("wxT", [I, H], F32R, kind="ExternalInput")
    whT = nc.dram_tensor("whT", [H, H], BF16, kind="ExternalInput")
    bias = nc.dram_tensor("bias", [128, KH], F32, kind="ExternalInput")
    out = nc.dram_tensor("out", [128, KH, BC], F32, kind="ExternalOutput")

    NT = t_steps * BC // 512  # number of 512-wide column chunks in phase 1

    with TileContext(nc) as tc:
        with tc.tile_pool(name="dram", bufs=1, space="DRAM") as dram_pool, \
             tc.tile_pool(name="weights", bufs=1) as wpool:
            xp_dram = dram_pool.tile([128, KH, t_steps * BC], F32)

            # Stationary weights for both phases, resident for the whole run.
            wx_sb = wpool.tile([128, KI, H], F32R)
            wh_sb = wpool.tile([128, KH, H], BF16)
            bias_sb = wpool.tile([128, KH], F32)
            for k in range(KI):
                nc.sync.dma_start(out=wx_sb[:, k, :], in_=wxT[k * 128:(k + 1) * 128, :])
            for k in range(KH):
                nc.sync.dma_start(out=wh_sb[:, k, :], in_=whT[k * 128:(k + 1) * 128, :])
            nc.sync.dma_start(out=bias_sb, in_=bias[:, :])

            # ---------------- Phase 1: xp = W_x^T.T @ x^T + bias ----------
            with tc.tile_pool(name="xin", bufs=2) as xpool, \
                 tc.tile_pool(name="ps1", bufs=2, space="PSUM") as ps1, \
                 tc.tile_pool(name="xpout", bufs=3) as xop:
                for n in range(NT):
                    xin = xpool.tile([128, KI, 512], F32R, tag="xin")
                    for k in range(KI):
                        nc.sync.dma_start(
                            out=xin[:, k, :],
                            in_=xT[k * 128:(k + 1) * 128, n * 512:(n + 1) * 512])
                    for m in range(KH):
                        psum = ps1.tile([128, 512], F32, tag="ps")
                        for k in range(KI):
                            nc.tensor.matmul(
                                psum,
                                lhsT=wx_sb[:, k, m * 128:(m + 1) * 128],
                                rhs=xin[:, k, :],
                                start=(k == 0), stop=(k == KI - 1))
                        xp_sb = xop.tile([128, 512], F32, tag="xp")
                        nc.scalar.activation(xp_sb, psum, AF.Identity,
                                             bias=bias_sb[:, m:m + 1])
                        nc.sync.dma_start(
                            out=xp_dram[:, m, n * 512:(n + 1) * 512], in_=xp_sb)

            # ---------------- Phase 2: the recurrence ---------------------
            with tc.tile_pool(name="hT", bufs=2) as hpool, \
                 tc.tile_pool(name="xpt", bufs=4) as xptp, \
                 tc.tile_pool(name="pre", bufs=2) as prep, \
                 tc.tile_pool(name="ps2", bufs=2, space="PSUM") as ps2, \
                 tc.tile_pool(name="fin", bufs=1) as finp:
                hT = hpool.tile([128, KH, BC], BF16, tag="hT")
                nc.vector.memset(hT, 0.0)
                for t in range(t_steps):
                    xp_t = xptp.tile([128, KH, BC], F32, tag="xpt")
                    nc.sync.dma_start(out=xp_t,
                                      in_=xp_dram[:, :, t * BC:(t + 1) * BC])
                    psum = ps2.tile([128, KH, BC], F32, tag="ps2")
                    for j in range(KH):
                        for k in range(KH):
                            nc.tensor.matmul(
                                psum[:, j, :],
                                lhsT=wh_sb[:, k, j * 128:(j + 1) * 128],
                                rhs=hT[:, k, :],
                                start=(k == 0), stop=(k == KH - 1))
                    pre = prep.tile([128, KH, BC], F32, tag="pre")
                    nc.vector.tensor_add(pre, psum, xp_t)
                    hT = hpool.tile([128, KH, BC], BF16, tag="hT")
                    nc.scalar.activation(hT, pre, AF.Tanh)
                    if t == t_steps - 1:
                        fin = finp.tile([128, KH, BC], F32)
                        nc.scalar.activation(fin, pre, AF.Tanh)
                        nc.sync.dma_start(out=out[:, :, :], in_=fin)

    nc.compile()
    return nc


def _get_built():
    global _BUILT
    if _BUILT is None:
        _BUILT = build(T)
    return _BUILT


def _prep_inputs(x_seq, W_h, b_h, W_x, b_x, t_steps=T):
    x_seq = np.asarray(x_seq, dtype=np.float32)
    W_h = np.asarray(W_h, dtype=np.float32)
    b_h = np.asarray(b_h, dtype=np.float32)
    W_x = np.asarray(W_x, dtype=np.float32)
    b_x = np.asarray(b_x, dtype=np.float32)

    wxT = np.ascontiguousarray(W_x.T)                      # [I, H] fp32
    whT = np.ascontiguousarray(W_h.T).astype(ml_dtypes.bfloat16)  # [H, H]
    bias = np.ascontiguousarray((b_x + b_h).reshape(KH, 128).T)   # [128, KH]

    in_maps = []
    for c in range(N_CORES):
        xs = x_seq[c * BC:(c + 1) * BC, :t_steps, :]       # [BC, t, I]
        xTc = np.ascontiguousarray(xs.transpose(2, 1, 0).reshape(I, t_steps * BC))
        in_maps.append({"xT": xTc, "wxT": wxT, "whT": whT, "bias": bias})
    return in_maps


def _assemble(results):
    outs = []
    for c in range(N_CORES):
        o = results[c]["out"]                              # [128, KH, BC]
        outs.append(o.transpose(2, 1, 0).reshape(BC, H))   # h = j*128 + p
    return np.concatenate(outs, axis=0).astype(np.float32)


def kernel(x_seq, W_h, b_h, W_x, b_x):
    nc = _get_built()
    in_maps = _prep_inputs(x_seq, W_h, b_h, W_x, b_x)
    res = run_bass_kernel_spmd(nc, in_maps, list(range(N_CORES)))
    return _assemble(res.results)


# revision 2
# speedup vs baseline: 1.2025x; 1.2025x over previous
"""Elman RNN cell (tanh) on 8 Trainium2 NeuronCores.

h_t = tanh(h_{t-1} @ W_h^T + b_h + x_t @ W_x^T + b_x), return h_T.

Strategy (hardcoded for B=64, T=512, I=H=1024, 8 cores):
  - Data parallel over batch: 8 batch elements per core. Weights replicated.
  - Phase 1 (per core): xp[h, t, b] = sum_i W_x[h,i] x[b,t,i] + (b_x+b_h)[h]
    as a tiled matmul with W_x^T stationary (bf16) and x^T moving, output
    written fp32 to a DRAM scratch in [p, m, t*8+b] layout (h = m*128+p).
  - Phase 2 (per core): 512 sequential steps. W_h^T stationary in bf16,
    h kept as hT[p, k, b] (h_in = k*128+p) so the matmul output
    [h_out partitions, batch] is directly the next hT (no transposes).
    The 8 output chunks are split across 4 PSUM banks in groups
    (0-2)(3-5)(6)(7) so the DVE add(+xp) and ACT tanh of early groups
    overlap the tail of the matmul burst; separate per-group h tiles
    keep cross-step dependencies exact.
"""

import os
import sys

if "/opt/trn_rl_repo" not in sys.path:
    sys.path.insert(0, "/opt/trn_rl_repo")

import numpy as np
import ml_dtypes

import concourse.bass as bass  # noqa: F401
import concourse.tile as tile
from concourse import bacc, mybir
from concourse.bass_utils import run_bass_kernel_spmd
from concourse.tile import TileContext

B, T, I, H = 64, 512, 1024, 1024
N_CORES = 8
BC = B // N_CORES  # batch per core = 8
KI = I // 128      # 8 k-chunks of the input dim
KH = H // 128      # 8 chunks of the hidden dim
F32 = mybir.dt.float32
BF16 = mybir.dt.bfloat16
AF = mybir.ActivationFunctionType

# Output-chunk grouping for phase 2: each group of j-chunks shares one PSUM
# bank and one h tile, so tanh(group) can run while later groups matmul.
GROUPS = [(0, 1, 2), (3, 4, 5), (6,), (7,)]

_BUILT = None


def build(t_steps: int = T):
    nc = bacc.Bacc("TRN2", target_bir_lowering=False, debug=False,
                   num_devices=N_CORES)

    # Per-core inputs (host pre-arranged).
    xT = nc.dram_tensor("xT", [I, t_steps * BC], BF16, kind="ExternalInput")
    wxT = nc.dram_tensor("wxT", [I, H], BF16, kind="ExternalInput")
    whT = nc.dram_tensor("whT", [H, H], BF16, kind="ExternalInput")
    bias = nc.dram_tensor("bias", [128, KH], F32, kind="ExternalInput")
    out = nc.dram_tensor("out", [128, KH, BC], F32, kind="ExternalOutput")

    NT = t_steps * BC // 512  # number of 512-wide column chunks in phase 1

    with TileContext(nc) as tc:
        with tc.tile_pool(name="dram", bufs=1, space="DRAM") as dram_pool, \
             tc.tile_pool(name="weights", bufs=1) as wpool:
            xp_dram = dram_pool.tile([128, KH, t_steps * BC], F32)

            # Stationary weights for both phases, resident for the whole run.
            wx_sb = wpool.tile([128, KI, H], BF16)
            wh_sb = wpool.tile([128, KH, H], BF16)
            bias_sb = wpool.tile([128, KH], F32)
            for k in range(KI):
                nc.sync.dma_start(out=wx_sb[:, k, :], in_=wxT[k * 128:(k + 1) * 128, :])
            for k in range(KH):
                nc.sync.dma_start(out=wh_sb[:, k, :], in_=whT[k * 128:(k + 1) * 128, :])
            nc.sync.dma_start(out=bias_sb, in_=bias[:, :])

            # ---------------- Phase 1: xp = W_x^T.T @ x^T + bias ----------
            with tc.tile_pool(name="xin", bufs=2) as xpool, \
                 tc.tile_pool(name="ps1", bufs=2, space="PSUM") as ps1, \
                 tc.tile_pool(name="xpout", bufs=3) as xop:
                for n in range(NT):
                    xin = xpool.tile([128, KI, 512], BF16, tag="xin")
                    for k in range(KI):
                        nc.sync.dma_start(
                            out=xin[:, k, :],
                            in_=xT[k * 128:(k + 1) * 128, n * 512:(n + 1) * 512])
                    for m in range(KH):
                        psum = ps1.tile([128, 512], F32, tag="ps")
                        for k in range(KI):
                            nc.tensor.matmul(
                                psum,
                                lhsT=wx_sb[:, k, m * 128:(m + 1) * 128],
                                rhs=xin[:, k, :],
                                start=(k == 0), stop=(k == KI - 1))
                        xp_sb = xop.tile([128, 512], F32, tag="xp")
                        nc.scalar.activation(xp_sb, psum, AF.Identity,
                                             bias=bias_sb[:, m:m + 1])
                        nc.sync.dma_start(
                            out=xp_dram[:, m, n * 512:(n + 1) * 512], in_=xp_sb)

            # ---------------- Phase 2: the recurrence ---------------------
            ngroups = len(GROUPS)
            with tc.tile_pool(name="hT0", bufs=2) as hp0, \
                 tc.tile_pool(name="hT1", bufs=2) as hp1, \
                 tc.tile_pool(name="hT2", bufs=2) as hp2, \
                 tc.tile_pool(name="hT3", bufs=2) as hp3, \
                 tc.tile_pool(name="xpt", bufs=6) as xptp, \
                 tc.tile_pool(name="pre0", bufs=2) as pp0, \
                 tc.tile_pool(name="pre1", bufs=2) as pp1, \
                 tc.tile_pool(name="pre2", bufs=2) as pp2, \
                 tc.tile_pool(name="pre3", bufs=2) as pp3, \
                 tc.tile_pool(name="ps2a", bufs=2, space="PSUM") as psa, \
                 tc.tile_pool(name="ps2b", bufs=2, space="PSUM") as psb, \
                 tc.tile_pool(name="ps2c", bufs=2, space="PSUM") as psc, \
                 tc.tile_pool(name="ps2d", bufs=2, space="PSUM") as psd, \
                 tc.tile_pool(name="fin", bufs=1) as finp:
                hpools = [hp0, hp1, hp2, hp3]
                ppools = [pp0, pp1, pp2, pp3]
                pspools = [psa, psb, psc, psd]

                # h state, one tile per group of j-chunks
                hts = []
                for g, js in enumerate(GROUPS):
                    ht = hpools[g].tile([128, len(js), BC], BF16, tag=f"h{g}")
                    nc.vector.memset(ht, 0.0)
                    hts.append(ht)

                def h_slice(k):
                    for g, js in enumerate(GROUPS):
                        if k in js:
                            return hts[g][:, js.index(k), :]
                    raise AssertionError

                fin = finp.tile([128, KH, BC], F32)
                for t in range(t_steps):
                    xp_t = xptp.tile([128, KH, BC], F32, tag="xpt")
                    nc.sync.dma_start(out=xp_t,
                                      in_=xp_dram[:, :, t * BC:(t + 1) * BC])
                    # matmul burst, group by group; tanh of finished groups
                    # overlaps the next group's matmuls
                    new_hts = [None] * ngroups
                    psums = []
                    for g, js in enumerate(GROUPS):
                        psum = pspools[g].tile([128, len(js), BC], F32,
                                               tag=f"ps{g}")
                        psums.append(psum)
                        for ji, j in enumerate(js):
                            for k in range(KH):
                                nc.tensor.matmul(
                                    psum[:, ji, :],
                                    lhsT=wh_sb[:, k, j * 128:(j + 1) * 128],
                                    rhs=h_slice(k),
                                    start=(k == 0), stop=(k == KH - 1))
                        j_lo, j_hi = js[0], js[-1] + 1
                        pre = ppools[g].tile([128, len(js), BC], F32,
                                             tag=f"pre{g}")
                        nc.vector.tensor_add(pre, psum,
                                             xp_t[:, j_lo:j_hi, :])
                        nh = hpools[g].tile([128, len(js), BC], BF16,
                                            tag=f"h{g}")
                        nc.scalar.activation(nh, pre, AF.Tanh)
                        new_hts[g] = nh
                        if t == t_steps - 1:
                            nc.scalar.activation(fin[:, j_lo:j_hi, :], pre,
                                                 AF.Tanh)
                    hts = new_hts
                nc.sync.dma_start(out=out[:, :, :], in_=fin)

    nc.compile()
    return nc


def _get_built():
    global _BUILT
    if _BUILT is None:
        _BUILT = build(T)
    return _BUILT


def _prep_inputs(x_seq, W_h, b_h, W_x, b_x, t_steps=T):
    x_seq = np.asarray(x_seq, dtype=np.float32)
    W_h = np.asarray(W_h, dtype=np.float32)
    b_h = np.asarray(b_h, dtype=np.float32)
    W_x = np.asarray(W_x, dtype=np.float32)
    b_x = np.asarray(b_x, dtype=np.float32)

    wxT = np.ascontiguousarray(W_x.T).astype(ml_dtypes.bfloat16)  # [I, H]
    whT = np.ascontiguousarray(W_h.T).astype(ml_dtypes.bfloat16)  # [H, H]
    bias = np.ascontiguousarray((b_x + b_h).reshape(KH, 128).T)   # [128, KH]

    in_maps = []
    for c in range(N_CORES):
        xs = x_seq[c * BC:(c + 1) * BC, :t_steps, :]       # [BC, t, I]
        xTc = np.ascontiguousarray(
            xs.transpose(2, 1, 0).reshape(I, t_steps * BC)).astype(
                ml_dtypes.bfloat16)
        in_maps.append({"xT": xTc, "wxT": wxT, "whT": whT, "bias": bias})
    return in_maps


def _assemble(results):
    outs = []
    for c in range(N_CORES):
        o = results[c]["out"]                              # [128, KH, BC]
        outs.append(o.transpose(2, 1, 0).reshape(BC, H))   # h = j*128 + p
    return np.concatenate(outs, axis=0).astype(np.float32)


def kernel(x_seq, W_h, b_h, W_x, b_x):
    nc = _get_built()
    in_maps = _prep_inputs(x_seq, W_h, b_h, W_x, b_x)
    res = run_bass_kernel_spmd(nc, in_maps, list(range(N_CORES)))
    return _assemble(res.results)
